# revision 1
# baseline (speedup 1.0000x reference)
"""GST-LSTM cell (graph-conv LSTM) on 8 Trainium2 NeuronCores.

Computation (reference):
    g  = adj_matrix @ Ht_1                       # (N, H)  -- dominant cost
    i  = sigmoid(ht @ Wxi.T + bxi + g @ Whi.T + bhi)
    f  = sigmoid(ht @ Wxf.T + bxf + g @ Whf.T + bhf)
    o  = sigmoid(ht @ Wxo.T + bxo + g @ Who.T + bho)
    u  = tanh   (ht @ Wxc.T + bxc + g @ Whc.T + bhc)
    Ct = f * Ct_1 + i * u
    Ht = o * tanh(Ct)

Sharding: node dim (rows of adj, ht, Ct_1; output rows) split across the
8 cores; Ht_1 replicated. No collectives needed.

Device layout: everything is computed feature-major ([64, nodes] tiles)
so that the PE contraction dim lands on partitions without any on-device
transposes:
  - adj is transposed + cast to fp16 on the host; each core streams its
    [16384, 2048] fp16 column-slice of adj^T and computes
    G^T[64, m] += H_k^T-stationary @ adjT-moving per 128-row k-tile,
    accumulating in PSUM over 128 k-tiles.
  - the eight 64x64 Linears run in fp32 (weights host-pre-transposed),
    also feature-major; biases enter via the ACT engine's per-partition
    bias operand together with the sigmoid/tanh.
  - gating is elementwise on DVE/ACT; outputs leave as [64, 2048] fp32
    and the host transposes them back.

fp16 for the adj @ Ht_1 product keeps end-to-end relative error at the
~2e-4 level (fp32 PSUM accumulation) while halving HBM traffic of the
1 GiB adjacency stream, which is what the memory-bound regime rewards.
"""

import numpy as np

N = 16384
D = 64
N_CORES = 8
ROWS = N // N_CORES          # 2048 nodes per core
MBW = 512                    # m-block width (PE moving free dim / PSUM bank)
MB = ROWS // MBW             # 4 m-blocks per core
KT = N // 128                # 128 k-tiles of 128 contraction rows
KTB = 4                      # k-tiles fetched per DMA (1 MiB stripes)
GD = KT // KTB               # 32 stripe DMAs per m-block

_GATE_FUNCS = ("Sigmoid", "Sigmoid", "Sigmoid", "Tanh")  # i, f, o, u


def _split_excess_waits(nc, max_waits=1):
    """Split >max_waits sem waits off instructions onto preceding NOPs.

    The walrus build here rejects instructions carrying more than a
    couple of sync waits ("Too many sync wait commands" from
    setupSyncWait during codegen). Tile's wait assignment doesn't know
    that limit; an NX engine executes its stream in order, so moving
    the excess waits onto same-engine NOPs directly before the
    instruction preserves ordering semantics with a legal encoding.
    """
    from concourse import mybir

    fn = nc.m.functions[0]
    for bb in fn.blocks:
        out = []
        for inst in bb.instructions:
            si = getattr(inst, "sync_info", None)
            if si is not None and si.on_wait and len(si.on_wait) > max_waits:
                waits = list(si.on_wait)
                spill, keep = waits[:-max_waits], waits[-max_waits:]
                for i in range(0, len(spill), max_waits):
                    nop = mybir.InstNoOp(
                        name=nc.get_next_instruction_name(),
                        sync_info=mybir.SyncInfo(
                            on_wait=spill[i:i + max_waits], on_update=[]
                        ),
                        bass_nofuse=True,
                        engine=inst.engine,
                    )
                    out.append(nop)
                si.on_wait = keep
            out.append(inst)
        bb.instructions[:] = out


RES_SCALE = 8192.0  # 2**13: fp8 residual stream pre-scale


def build(n=N, rows=ROWS, mbw=MBW, ktb=KTB, repeat=1, adj_bufs=4,
          split_waits=True, fp8_res=False):
    """Build the per-core Bass program. All cores run the same program."""
    import concourse.bass as bass
    import concourse.mybir as mybir
    from concourse import tile

    dt = mybir.dt
    act = mybir.ActivationFunctionType
    f16, f32, f8 = dt.float16, dt.float32, dt.float8e4
    alu = mybir.AluOpType

    kt = n // 128
    mb = rows // mbw
    gd = kt // ktb

    nc = bass.Bass()
    adjt = nc.declare_dram_parameter("adjt", [n, rows], f16, isOutput=False)
    # hh packs [H_hi | H_lo] fp16 side by side per k-tile: one [128,128]
    # stationary computes both products in a single matmul pass (PSUM rows
    # 0:64 accumulate A@H_hi, rows 64:128 accumulate A@H_lo); the moving
    # operand streams once, so the lo-correction is free on the PE.
    hh = nc.declare_dram_parameter("hh", [128, kt * 2 * D], f16, isOutput=False)
    if fp8_res:
        # fp8 stream of (adj' - fp16(adj')) * RES_SCALE and fp8(H_hi):
        # removes the adjacency-side fp16 quantization error.
        adjr = nc.declare_dram_parameter("adjr", [n, rows], f8, isOutput=False)
        hh8 = nc.declare_dram_parameter("hh8", [128, kt * D], f8, isOutput=False)
    xt = nc.declare_dram_parameter("xt", [D, rows], f32, isOutput=False)
    ct = nc.declare_dram_parameter("ct", [D, rows], f32, isOutput=False)
    wxt = nc.declare_dram_parameter("wxt", [D, 4 * D], f32, isOutput=False)
    wht = nc.declare_dram_parameter("wht", [D, 4 * D], f32, isOutput=False)
    bias = nc.declare_dram_parameter("bias", [D, 4], f32, isOutput=False)
    ht_out = nc.declare_dram_parameter("ht_out", [D, rows], f32, isOutput=True)
    ct_out = nc.declare_dram_parameter("ct_out", [D, rows], f32, isOutput=True)

    adjt_r = adjt[:].rearrange(
        "(g a p) (mb mj) -> g mb p a mj", a=ktb, p=128, mj=mbw
    )
    if fp8_res:
        adjr_r = adjr[:].rearrange(
            "(g a p) (mb mj) -> g mb p a mj", a=ktb, p=128, mj=mbw
        )

    with tile.TileContext(nc) as tc:
        with (
            tc.tile_pool(name="const", bufs=1) as cst,
            tc.tile_pool(name="adj", bufs=adj_bufs) as apool,
            tc.tile_pool(name="b64", bufs=(3 if mbw <= 512 else 2)) as b64,
            tc.tile_pool(name="gpsum", bufs=2, space="PSUM") as gpsum,
            tc.tile_pool(
                name="gatepsum", bufs=(3 if mbw <= 512 else 2), space="PSUM"
            ) as gatepsum,
        ):
            hh_sb = cst.tile([128, kt * 2 * D], f16)
            nc.sync.dma_start(hh_sb[:], hh[:])
            if fp8_res:
                hh8_sb = cst.tile([128, kt * D], f8)
                nc.sync.dma_start(hh8_sb[:], hh8[:])
            wxt_sb = cst.tile([D, 4 * D], f32)
            nc.sync.dma_start(wxt_sb[:], wxt[:])
            wht_sb = cst.tile([D, 4 * D], f32)
            nc.sync.dma_start(wht_sb[:], wht[:])
            bias_sb = cst.tile([D, 4], f32)
            nc.sync.dma_start(bias_sb[:], bias[:])
            xt_sb = cst.tile([D, rows], f32)
            nc.sync.dma_start(xt_sb[:], xt[:])
            ct_sb = cst.tile([D, rows], f32)
            nc.sync.dma_start(ct_sb[:], ct[:])

            def body(_iv=None):
                for mbi in range(mb):
                    mbs = slice(mbi * mbw, (mbi + 1) * mbw)
                    gps = gpsum.tile([128, mbw], f32, tag="gps")
                    if fp8_res:
                        grs = gpsum.tile([D, mbw], f32, tag="grs")
                    for g in range(gd):
                        stripe = apool.tile([128, ktb * mbw], f16, tag="stripe")
                        nc.sync.dma_start(stripe[:], adjt_r[g, mbi])
                        if fp8_res:
                            stripe8 = apool.tile(
                                [128, ktb * mbw], f8, tag="stripe8"
                            )
                            nc.sync.dma_start(stripe8[:], adjr_r[g, mbi])
                        for a in range(ktb):
                            kti = g * ktb + a
                            nc.tensor.matmul(
                                gps[:],
                                hh_sb[:, kti * 2 * D:(kti + 1) * 2 * D],
                                stripe[:, a * mbw:(a + 1) * mbw],
                                start=(kti == 0),
                                stop=(kti == kt - 1),
                            )
                            if fp8_res:
                                nc.tensor.matmul(
                                    grs[:],
                                    hh8_sb[:, kti * D:(kti + 1) * D],
                                    stripe8[:, a * mbw:(a + 1) * mbw],
                                    start=(kti == 0),
                                    stop=(kti == kt - 1),
                                )
                    gtb = b64.tile([D, mbw], f32, tag="gtb")
                    nc.vector.tensor_copy(gtb[:], gps[0:D, :])
                    nc.vector.tensor_add(gtb[:], gtb[:], gps[D:2 * D, :])
                    if fp8_res:
                        nc.vector.scalar_tensor_tensor(
                            gtb[:], grs[:], 1.0 / RES_SCALE, gtb[:],
                            alu.mult, alu.add,
                        )

                    gates = []
                    for gi, fname in enumerate(_GATE_FUNCS):
                        pg = gatepsum.tile([D, mbw], f32, tag="pg")
                        nc.tensor.matmul(
                            pg[:],
                            wxt_sb[:, gi * D:(gi + 1) * D],
                            xt_sb[:, mbs],
                            start=True,
                            stop=False,
                        )
                        nc.tensor.matmul(
                            pg[:],
                            wht_sb[:, gi * D:(gi + 1) * D],
                            gtb[:],
                            start=False,
                            stop=True,
                        )
                        gate_sb = b64.tile([D, mbw], f32, tag=f"gate{gi}")
                        nc.scalar.activation(
                            gate_sb[:],
                            pg[:],
                            getattr(act, fname),
                            bias=bias_sb[:, gi:gi + 1],
                        )
                        gates.append(gate_sb)
                    it_, ft_, ot_, ut_ = gates

                    t1 = b64.tile([D, mbw], f32, tag="t1")
                    nc.vector.tensor_mul(t1[:], ft_[:], ct_sb[:, mbs])
                    t2 = b64.tile([D, mbw], f32, tag="t2")
                    nc.vector.tensor_mul(t2[:], it_[:], ut_[:])
                    ctn = b64.tile([D, mbw], f32, tag="ctn")
                    nc.vector.tensor_add(ctn[:], t1[:], t2[:])
                    nc.sync.dma_start(ct_out[:, mbs], ctn[:])
                    tct = b64.tile([D, mbw], f32, tag="tct")
                    nc.scalar.activation(tct[:], ctn[:], act.Tanh)
                    htn = b64.tile([D, mbw], f32, tag="htn")
                    nc.vector.tensor_mul(htn[:], ot_[:], tct[:])
                    nc.sync.dma_start(ht_out[:, mbs], htn[:])

            if repeat == 1:
                body()
            else:
                with tc.For_i(0, repeat, 1) as _i:
                    body(_i)

    if split_waits:
        _split_excess_waits(nc)
    return nc


def make_in_maps(inputs, n=N, n_cores=N_CORES, fp8_res=False):
    """Host-side sharding + relayout. Returns per-core input dicts."""
    import ml_dtypes
    rows = n // n_cores
    kt = n // 128
    adj = np.asarray(inputs["adj_matrix"], dtype=np.float32)
    H = np.asarray(inputs["Ht_1"], dtype=np.float32)
    ht = np.asarray(inputs["ht"], dtype=np.float32)
    Ct_1 = np.asarray(inputs["Ct_1"], dtype=np.float32)

    # H = hi + lo to ~2^-22: the hi/lo fp16 pair is packed side by side
    # per k-tile ([128, kt*128]) so one matmul computes both products.
    Hh32 = H.astype(np.float16).astype(np.float32)
    packed = np.empty((128, kt, 2 * D), dtype=np.float16)
    packed[:, :, :D] = Hh32.reshape(kt, 128, D).transpose(1, 0, 2)
    packed[:, :, D:] = (H - Hh32).reshape(kt, 128, D).transpose(1, 0, 2)
    hh = np.ascontiguousarray(packed.reshape(128, kt * 2 * D))
    if fp8_res:
        hh8 = np.ascontiguousarray(
            Hh32.reshape(kt, 128, D).transpose(1, 0, 2).reshape(128, kt * D)
        ).astype(ml_dtypes.float8_e4m3)

    gate_w = ("Wxi", "Wxf", "Wxo", "Wxc")
    gate_h = ("Whi", "Whf", "Who", "Whc")
    wxt = np.concatenate(
        [np.asarray(inputs[g + "_w"], np.float32).T for g in gate_w], axis=1
    )
    wht = np.concatenate(
        [np.asarray(inputs[g + "_w"], np.float32).T for g in gate_h], axis=1
    )
    # adj is shifted by -0.5 before the fp16 cast (halves quantization
    # error for uniform(0,1) entries). g = (adj-0.5)@H + 0.5*colsum(H)
    # broadcast over rows; the second term passes through the h-side
    # Linear as a per-feature constant, folded into the gate bias here.
    colsum = H.astype(np.float64).sum(axis=0)
    bias = np.stack(
        [
            np.asarray(inputs[gx + "_b"], np.float64)
            + np.asarray(inputs[gh + "_b"], np.float64)
            + 0.5 * (np.asarray(inputs[gh + "_w"], np.float64) @ colsum)
            for gx, gh in zip(gate_w, gate_h)
        ],
        axis=1,
    ).astype(np.float32)
    wxt = np.ascontiguousarray(wxt)
    wht = np.ascontiguousarray(wht)
    bias = np.ascontiguousarray(bias)

    in_maps = []
    for c in range(n_cores):
        rs = slice(c * rows, (c + 1) * rows)
        adjt_c = np.ascontiguousarray(adj[rs].T)
        adjt_c -= np.float32(0.5)
        adjt_c16 = adjt_c.astype(np.float16)
        in_maps.append(
            {
                "adjt": adjt_c16,
                "hh": hh,
                "xt": np.ascontiguousarray(ht[rs].T),
                "ct": np.ascontiguousarray(Ct_1[rs].T),
                "wxt": wxt,
                "wht": wht,
                "bias": bias,
            }
        )
        if fp8_res:
            res = adjt_c
            res -= adjt_c16.astype(np.float32)
            res *= np.float32(RES_SCALE)
            in_maps[-1]["adjr"] = res.astype(ml_dtypes.float8_e4m3)
            in_maps[-1]["hh8"] = hh8
    return in_maps


def gather(results):
    Ht = np.concatenate([r["ht_out"].T for r in results], axis=0)
    Ct = np.concatenate([r["ct_out"].T for r in results], axis=0)
    return np.ascontiguousarray(Ht), np.ascontiguousarray(Ct)


_PROGRAM_CACHE = {}


def kernel(**inputs):
    from concourse.bass_utils import run_bass_kernel_spmd

    if "nc" not in _PROGRAM_CACHE:
        _PROGRAM_CACHE["nc"] = build()
    nc = _PROGRAM_CACHE["nc"]
    in_maps = make_in_maps(inputs)
    res = run_bass_kernel_spmd(nc, in_maps, list(range(N_CORES)))
    return gather(res.results)



# revision 12
# speedup vs baseline: 1.5577x; 1.5577x over previous
"""GST-LSTM cell (graph-conv LSTM) on 8 Trainium2 NeuronCores.

Computation (reference):
    g  = adj_matrix @ Ht_1                       # (N, H)  -- dominant cost
    i  = sigmoid(ht @ Wxi.T + bxi + g @ Whi.T + bhi)
    f  = sigmoid(ht @ Wxf.T + bxf + g @ Whf.T + bhf)
    o  = sigmoid(ht @ Wxo.T + bxo + g @ Who.T + bho)
    u  = tanh   (ht @ Wxc.T + bxc + g @ Whc.T + bhc)
    Ct = f * Ct_1 + i * u
    Ht = o * tanh(Ct)

Sharding: node dim (rows of adj, ht, Ct_1; output rows) split across the
8 cores; Ht_1 replicated. No collectives needed.

Device layout: everything is computed feature-major ([64, nodes] tiles)
so that the PE contraction dim lands on partitions without any on-device
transposes:
  - adj is transposed, shifted by -0.5, cast to fp16 and PERMUTED on the
    host into stripe-contiguous order: each [128 k-rows, KTB*512] stripe
    a DMA fetches is one contiguous 1 MiB region of DRAM. Stripe loads
    alternate between the two HWDGE queues (SP via nc.sync, ACT via
    nc.scalar) so descriptor generation and completion latencies overlap.
  - Ht_1 enters as an fp16 hi/lo pair packed side by side per k-tile
    ([128, kt*128]): one [128,128] stationary computes both products in a
    single matmul pass (PSUM rows 0:64 accumulate A@H_hi, rows 64:128
    A@H_lo). It is loaded in 8 chunks so the first matmul starts ~2 us in.
  - the eight 64x64 Linears run feature-major; the x-side (ht) matmuls
    are fp16 (negligible error) and hoisted to m-block start so only the
    fp32 h-side matmul + activation + gating sit on the tail. Biases
    enter via the ACT engine's per-partition bias operand.
  - const loads and output stores ride the Pool SWDGE queue so they never
    head-of-line block the adjacency stream on the HWDGE rings.

fp16 for the adj @ Ht_1 product keeps end-to-end relative error at the
~1.2e-2 level (fp32 PSUM accumulation) while halving HBM traffic of the
1 GiB adjacency stream, which is what the memory-bound regime rewards.
"""

import numpy as np

N = 16384
D = 64
N_CORES = 8
ROWS = N // N_CORES          # 2048 nodes per core
MBW = 512                    # m-block width (PE moving free dim / PSUM bank)
MB = ROWS // MBW             # 4 m-blocks per core
KT = N // 128                # 128 k-tiles of 128 contraction rows
KTB = 8                      # k-tiles fetched per DMA (1 MiB stripes)
GD = KT // KTB               # stripe DMAs per m-block
HHC = 8                      # hh load chunks

_GATE_FUNCS = ("Sigmoid", "Sigmoid", "Sigmoid", "Tanh")  # i, f, o, u


def _split_excess_waits(nc, max_waits=1):
    """Split >max_waits sem waits off instructions onto preceding NOPs.

    The walrus build here rejects instructions carrying more than a
    couple of sync waits ("Too many sync wait commands" from
    setupSyncWait during codegen). Tile's wait assignment doesn't know
    that limit; an NX engine executes its stream in order, so moving
    the excess waits onto same-engine NOPs directly before the
    instruction preserves ordering semantics with a legal encoding.
    """
    from concourse import mybir

    fn = nc.m.functions[0]
    for bb in fn.blocks:
        out = []
        for inst in bb.instructions:
            si = getattr(inst, "sync_info", None)
            if si is not None and si.on_wait and len(si.on_wait) > max_waits:
                waits = list(si.on_wait)
                spill, keep = waits[:-max_waits], waits[-max_waits:]
                for i in range(0, len(spill), max_waits):
                    nop = mybir.InstNoOp(
                        name=nc.get_next_instruction_name(),
                        sync_info=mybir.SyncInfo(
                            on_wait=spill[i:i + max_waits], on_update=[]
                        ),
                        bass_nofuse=True,
                        engine=inst.engine,
                    )
                    out.append(nop)
                si.on_wait = keep
            out.append(inst)
        bb.instructions[:] = out


def build(n=N, rows=ROWS, mbw=MBW, ktb=KTB, repeat=1, adj_bufs=6,
          split_waits=True, dual_queue=True, hoist_x=True, pool_io=True,
          unroll=1):
    """Build the per-core Bass program. All cores run the same program."""
    import concourse.bass as bass
    import concourse.mybir as mybir
    from concourse import tile

    dt = mybir.dt
    act = mybir.ActivationFunctionType
    f16, f32 = dt.float16, dt.float32

    kt = n // 128
    mb = rows // mbw
    gd = kt // ktb
    hkc = kt // HHC              # k-tiles per hh chunk

    nc = bass.Bass()
    # stripe-contiguous: row block (mbi*gd + g)*128 .. +128 is one stripe
    adjt = nc.declare_dram_parameter("adjt", [mb * gd * 128, ktb * mbw], f16,
                                     isOutput=False)
    # hh packs [H_hi | H_lo] fp16 side by side per k-tile: one [128,128]
    # stationary computes both products in a single matmul pass.
    hh = nc.declare_dram_parameter("hh", [128, kt * 2 * D], f16, isOutput=False)
    xt = nc.declare_dram_parameter("xt", [D, rows], f16, isOutput=False)
    ct = nc.declare_dram_parameter("ct", [D, rows], f32, isOutput=False)
    wxt = nc.declare_dram_parameter("wxt", [D, 4 * D], f16, isOutput=False)
    wht = nc.declare_dram_parameter("wht", [D, 4 * D], f32, isOutput=False)
    bias = nc.declare_dram_parameter("bias", [D, 4], f32, isOutput=False)
    ht_out = nc.declare_dram_parameter("ht_out", [D, rows], f32, isOutput=True)
    ct_out = nc.declare_dram_parameter("ct_out", [D, rows], f32, isOutput=True)

    dma_engs = []

    with tile.TileContext(nc) as tc:
        with (
            tc.tile_pool(name="const", bufs=1) as cst,
            tc.tile_pool(name="adj", bufs=adj_bufs) as apool,
            tc.tile_pool(name="b64", bufs=3) as b64,
            tc.tile_pool(name="gpsum", bufs=2, space="PSUM") as gpsum,
            tc.tile_pool(name="gatepsum", bufs=1, space="PSUM") as gatepsum,
        ):
            io_eng = nc.scalar if pool_io else nc.sync
            dma_engs = [nc.sync, nc.scalar] if dual_queue else [nc.sync]

            # hh in chunks on the ACT ring (so SP's first job is stripe 0),
            # consts on Pool
            hh_eng = dma_engs[-1]
            hh_sb = []
            for c in range(HHC):
                t = cst.tile([128, hkc * 2 * D], f16, tag=f"hh{c}")
                hh_eng.dma_start(
                    t[:], hh[:, c * hkc * 2 * D:(c + 1) * hkc * 2 * D]
                )
                hh_sb.append(t)
            wxt_sb = cst.tile([D, 4 * D], f16)
            io_eng.dma_start(wxt_sb[:], wxt[:])
            wht_sb = cst.tile([D, 4 * D], f32)
            io_eng.dma_start(wht_sb[:], wht[:])
            bias_sb = cst.tile([D, 4], f32)
            io_eng.dma_start(bias_sb[:], bias[:])
            xt_sb = cst.tile([D, rows], f16)
            io_eng.dma_start(xt_sb[:], xt[:])
            ct_sb = cst.tile([D, rows], f32)
            io_eng.dma_start(ct_sb[:], ct[:])

            def body(_iv=None):
                # outputs of m-block i are issued mid-way through m-block
                # i+1's stripe stream: by then they are computed, so they
                # never head-of-line block the adjacency DMA rings.
                pending = []

                for mbi in range(mb):
                    mbs = slice(mbi * mbw, (mbi + 1) * mbw)

                    # x-side gate matmuls first: no dependence on the
                    # adjacency stream, so they fill PE while DMA runs and
                    # leave only the h-side matmul on the m-block tail.
                    pgs = []
                    for gi in range(4):
                        pg = gatepsum.tile([D, mbw], f32, tag=f"pg{gi}")
                        nc.tensor.matmul(
                            pg[:],
                            wxt_sb[:, gi * D:(gi + 1) * D],
                            xt_sb[:, mbs],
                            start=True,
                            stop=False,
                        )
                        pgs.append(pg)

                    gps = gpsum.tile([128, mbw], f32, tag="gps")
                    for g in range(gd):
                        # first stripes of an m-block ride SP while the ACT
                        # ring drains the previous tail's output stores
                        eng = (dma_engs[0] if (mbi == 0 and g < 4)
                               else dma_engs[g % len(dma_engs)])
                        stripe = apool.tile([128, ktb * mbw], f16, tag="stripe")
                        rb = (mbi * gd + g) * 128
                        eng.dma_start(stripe[:], adjt[rb:rb + 128, :])
                        if g == 6:
                            for pi, (dst, tsb) in enumerate(pending):
                                dma_engs[pi % len(dma_engs)].dma_start(dst, tsb[:])
                            pending.clear()
                        for a in range(ktb):
                            kti = g * ktb + a
                            hc, ho = kti // hkc, kti % hkc
                            nc.tensor.matmul(
                                gps[:],
                                hh_sb[hc][:, ho * 2 * D:(ho + 1) * 2 * D],
                                stripe[:, a * mbw:(a + 1) * mbw],
                                start=(kti == 0),
                                stop=(kti == kt - 1),
                            )
                    # tail in two column-halves so PE/ACT/DVE stages pipeline
                    hw = mbw // 2
                    for h in range(2):
                        hs = slice(h * hw, (h + 1) * hw)       # within m-block
                        hg = slice(mbi * mbw + h * hw, mbi * mbw + (h + 1) * hw)
                        gtb = b64.tile([D, hw], f32, tag=f"gtb{h}")
                        nc.vector.tensor_copy(gtb[:], gps[0:D, hs])
                        nc.vector.tensor_add(gtb[:], gtb[:], gps[D:2 * D, hs])
                        gates = []
                        for gi, fname in enumerate(_GATE_FUNCS):
                            pg = pgs[gi]
                            nc.tensor.matmul(
                                pg[:, hs],
                                wht_sb[:, gi * D:(gi + 1) * D],
                                gtb[:],
                                start=False,
                                stop=(h == 1),
                                skip_group_check=True,
                            )
                            gate_sb = b64.tile([D, hw], f32, tag=f"gate{gi}{h}")
                            nc.scalar.activation(
                                gate_sb[:],
                                pg[:, hs],
                                getattr(act, fname),
                                bias=bias_sb[:, gi:gi + 1],
                            )
                            gates.append(gate_sb)
                        it_, ft_, ot_, ut_ = gates

                        t1 = b64.tile([D, hw], f32, tag=f"t1{h}")
                        nc.vector.tensor_mul(t1[:], ft_[:], ct_sb[:, hg])
                        t2 = b64.tile([D, hw], f32, tag=f"t2{h}")
                        nc.vector.tensor_mul(t2[:], it_[:], ut_[:])
                        ctn = b64.tile([D, hw], f32, tag=f"ctn{h}")
                        nc.vector.tensor_add(ctn[:], t1[:], t2[:])
                        tct = b64.tile([D, hw], f32, tag=f"tct{h}")
                        nc.scalar.activation(tct[:], ctn[:], act.Tanh)
                        htn = b64.tile([D, hw], f32, tag=f"htn{h}")
                        nc.vector.tensor_mul(htn[:], ot_[:], tct[:])
                        if mbi == mb - 1:
                            io_eng.dma_start(ct_out[:, hg], ctn[:])
                            io_eng.dma_start(ht_out[:, hg], htn[:])
                        else:
                            pending.append((ct_out[:, hg], ctn))
                            pending.append((ht_out[:, hg], htn))

            if repeat == 1:
                for _ in range(unroll):
                    body()
            else:
                with tc.For_i(0, repeat, 1) as _i:
                    body(_i)

    if split_waits:
        _split_excess_waits(nc)
    return nc


def make_in_maps(inputs, n=N, n_cores=N_CORES):
    """Host-side sharding + relayout. Returns per-core input dicts."""
    rows = n // n_cores
    kt = n // 128
    mb = rows // MBW
    gd = kt // KTB
    adj = np.asarray(inputs["adj_matrix"], dtype=np.float32)
    H = np.asarray(inputs["Ht_1"], dtype=np.float32)
    ht = np.asarray(inputs["ht"], dtype=np.float32)
    Ct_1 = np.asarray(inputs["Ct_1"], dtype=np.float32)

    # H = hi + lo to ~2^-22: the hi/lo fp16 pair is packed side by side
    # per k-tile ([128, kt*128]) so one matmul computes both products.
    Hh32 = H.astype(np.float16).astype(np.float32)
    packed = np.empty((128, kt, 2 * D), dtype=np.float16)
    packed[:, :, :D] = Hh32.reshape(kt, 128, D).transpose(1, 0, 2)
    packed[:, :, D:] = (H - Hh32).reshape(kt, 128, D).transpose(1, 0, 2)
    hh = np.ascontiguousarray(packed.reshape(128, kt * 2 * D))

    gate_w = ("Wxi", "Wxf", "Wxo", "Wxc")
    gate_h = ("Whi", "Whf", "Who", "Whc")
    wxt = np.concatenate(
        [np.asarray(inputs[g + "_w"], np.float32).T for g in gate_w], axis=1
    ).astype(np.float16)
    wht = np.concatenate(
        [np.asarray(inputs[g + "_w"], np.float32).T for g in gate_h], axis=1
    )
    # adj is shifted by -0.5 before the fp16 cast (halves quantization
    # error for uniform(0,1) entries). g = (adj-0.5)@H + 0.5*colsum(H)
    # broadcast over rows; the second term passes through the h-side
    # Linear as a per-feature constant, folded into the gate bias here.
    colsum = H.astype(np.float64).sum(axis=0)
    bias = np.stack(
        [
            np.asarray(inputs[gx + "_b"], np.float64)
            + np.asarray(inputs[gh + "_b"], np.float64)
            + 0.5 * (np.asarray(inputs[gh + "_w"], np.float64) @ colsum)
            for gx, gh in zip(gate_w, gate_h)
        ],
        axis=1,
    ).astype(np.float32)
    wxt = np.ascontiguousarray(wxt)
    wht = np.ascontiguousarray(wht)
    bias = np.ascontiguousarray(bias)

    in_maps = []
    for c in range(n_cores):
        rs = slice(c * rows, (c + 1) * rows)
        adjt_c = np.ascontiguousarray(adj[rs].T)
        adjt_c -= np.float32(0.5)
        a16 = adjt_c.astype(np.float16)          # [n, rows]
        # stripe-contiguous permute: [(mb gd) 128, ktb*mbw] where the row
        # block (mbi*gd+g)*128 holds k-rows (g*ktb .. )*128 interleaved as
        # [p, a, mj] -> flat [128, ktb*mbw] for m-cols mbi*mbw..+mbw.
        a5 = a16.reshape(gd, KTB, 128, mb, MBW).transpose(3, 0, 2, 1, 4)
        adjt_s = np.ascontiguousarray(a5.reshape(mb * gd * 128, KTB * MBW))
        in_maps.append(
            {
                "adjt": adjt_s,
                "hh": hh,
                "xt": np.ascontiguousarray(ht[rs].T).astype(np.float16),
                "ct": np.ascontiguousarray(Ct_1[rs].T),
                "wxt": wxt,
                "wht": wht,
                "bias": bias,
            }
        )
    return in_maps


def gather(results):
    Ht = np.concatenate([r["ht_out"].T for r in results], axis=0)
    Ct = np.concatenate([r["ct_out"].T for r in results], axis=0)
    return np.ascontiguousarray(Ht), np.ascontiguousarray(Ct)


_PROGRAM_CACHE = {}


def kernel(**inputs):
    from concourse.bass_utils import run_bass_kernel_spmd

    if "nc" not in _PROGRAM_CACHE:
        _PROGRAM_CACHE["nc"] = build()
    nc = _PROGRAM_CACHE["nc"]
    in_maps = make_in_maps(inputs)
    res = run_bass_kernel_spmd(nc, in_maps, list(range(N_CORES)))
    return gather(res.results)


# revision 13
# speedup vs baseline: 1.6288x; 1.0457x over previous
"""GST-LSTM cell (graph-conv LSTM) on 8 Trainium2 NeuronCores.

Computation (reference):
    g  = adj_matrix @ Ht_1                       # (N, H)  -- dominant cost
    i  = sigmoid(ht @ Wxi.T + bxi + g @ Whi.T + bhi)
    f  = sigmoid(ht @ Wxf.T + bxf + g @ Whf.T + bhf)
    o  = sigmoid(ht @ Wxo.T + bxo + g @ Who.T + bho)
    u  = tanh   (ht @ Wxc.T + bxc + g @ Whc.T + bhc)
    Ct = f * Ct_1 + i * u
    Ht = o * tanh(Ct)

Sharding: node dim (rows of adj, ht, Ct_1; output rows) split across the
8 cores; Ht_1 replicated. No collectives needed.

Device layout: everything is computed feature-major ([64, nodes] tiles)
so that the PE contraction dim lands on partitions without any on-device
transposes:
  - adj is transposed, shifted by -0.5, cast to fp16 and PERMUTED on the
    host into stripe-contiguous order: each [128 k-rows, KTB*512] stripe
    a DMA fetches is one contiguous 1 MiB region of DRAM. Stripe loads
    alternate between the two HWDGE queues (SP via nc.sync, ACT via
    nc.scalar) so descriptor generation and completion latencies overlap.
  - Ht_1 enters as an fp16 hi/lo pair packed side by side per k-tile
    ([128, kt*128]): one [128,128] stationary computes both products in a
    single matmul pass (PSUM rows 0:64 accumulate A@H_hi, rows 64:128
    A@H_lo). It is loaded in 8 chunks so the first matmul starts ~2 us in.
  - the eight 64x64 Linears run feature-major; the x-side (ht) matmuls
    are fp16 (negligible error) and hoisted to m-block start so only the
    fp32 h-side matmul + activation + gating sit on the tail. Biases
    enter via the ACT engine's per-partition bias operand.
  - const loads and output stores ride the Pool SWDGE queue so they never
    head-of-line block the adjacency stream on the HWDGE rings.

fp16 for the adj @ Ht_1 product keeps end-to-end relative error at the
~1.2e-2 level (fp32 PSUM accumulation) while halving HBM traffic of the
1 GiB adjacency stream, which is what the memory-bound regime rewards.
"""

import numpy as np

N = 16384
D = 64
N_CORES = 8
ROWS = N // N_CORES          # 2048 nodes per core
MBW = 512                    # m-block width (PE moving free dim / PSUM bank)
MB = ROWS // MBW             # 4 m-blocks per core
KT = N // 128                # 128 k-tiles of 128 contraction rows
KTB = 8                      # k-tiles fetched per DMA (1 MiB stripes)
GD = KT // KTB               # stripe DMAs per m-block
HHC = 8                      # hh load chunks

_GATE_FUNCS = ("Sigmoid", "Sigmoid", "Sigmoid", "Tanh")  # i, f, o, u


def _split_excess_waits(nc, max_waits=1):
    """Split >max_waits sem waits off instructions onto preceding NOPs.

    The walrus build here rejects instructions carrying more than a
    couple of sync waits ("Too many sync wait commands" from
    setupSyncWait during codegen). Tile's wait assignment doesn't know
    that limit; an NX engine executes its stream in order, so moving
    the excess waits onto same-engine NOPs directly before the
    instruction preserves ordering semantics with a legal encoding.
    """
    from concourse import mybir

    fn = nc.m.functions[0]
    for bb in fn.blocks:
        out = []
        for inst in bb.instructions:
            si = getattr(inst, "sync_info", None)
            if si is not None and si.on_wait and len(si.on_wait) > max_waits:
                waits = list(si.on_wait)
                spill, keep = waits[:-max_waits], waits[-max_waits:]
                for i in range(0, len(spill), max_waits):
                    nop = mybir.InstNoOp(
                        name=nc.get_next_instruction_name(),
                        sync_info=mybir.SyncInfo(
                            on_wait=spill[i:i + max_waits], on_update=[]
                        ),
                        bass_nofuse=True,
                        engine=inst.engine,
                    )
                    out.append(nop)
                si.on_wait = keep
            out.append(inst)
        bb.instructions[:] = out


def build(n=N, rows=ROWS, mbw=MBW, ktb=KTB, repeat=1, adj_bufs=6,
          split_waits=True, dual_queue=True, hoist_x=True, pool_io=True,
          unroll=1):
    """Build the per-core Bass program. All cores run the same program."""
    import concourse.bass as bass
    import concourse.mybir as mybir
    from concourse import tile

    dt = mybir.dt
    act = mybir.ActivationFunctionType
    f16, f32 = dt.float16, dt.float32

    kt = n // 128
    mb = rows // mbw
    gd = kt // ktb
    hkc = kt // HHC              # k-tiles per hh chunk

    nc = bass.Bass()
    # stripe-contiguous: row block (mbi*gd + g)*128 .. +128 is one stripe
    adjt = nc.declare_dram_parameter("adjt", [mb * gd * 128, ktb * mbw], f16,
                                     isOutput=False)
    # hh packs [H_hi | H_lo] fp16 side by side per k-tile: one [128,128]
    # stationary computes both products in a single matmul pass.
    hh = nc.declare_dram_parameter("hh", [128, kt * 2 * D], f16, isOutput=False)
    xt = nc.declare_dram_parameter("xt", [D, rows], f16, isOutput=False)
    ct = nc.declare_dram_parameter("ct", [D, rows], f32, isOutput=False)
    wxt = nc.declare_dram_parameter("wxt", [D, 4 * D], f16, isOutput=False)
    wht = nc.declare_dram_parameter("wht", [D, 4 * D], f32, isOutput=False)
    bias = nc.declare_dram_parameter("bias", [D, 4], f32, isOutput=False)
    ht_out = nc.declare_dram_parameter("ht_out", [D, rows], f32, isOutput=True)
    ct_out = nc.declare_dram_parameter("ct_out", [D, rows], f32, isOutput=True)

    dma_engs = []

    with tile.TileContext(nc) as tc:
        with (
            tc.tile_pool(name="const", bufs=1) as cst,
            tc.tile_pool(name="adj", bufs=adj_bufs) as apool,
            tc.tile_pool(name="b64", bufs=3) as b64,
            tc.tile_pool(name="gpsum", bufs=2, space="PSUM") as gpsum,
            tc.tile_pool(name="gatepsum", bufs=1, space="PSUM") as gatepsum,
        ):
            io_eng = nc.scalar if pool_io else nc.sync
            dma_engs = [nc.sync, nc.scalar] if dual_queue else [nc.sync]

            # hh in chunks on the ACT ring (so SP's first job is stripe 0),
            # consts on Pool
            hh_eng = dma_engs[-1]
            hh_sb = []
            for c in range(HHC):
                t = cst.tile([128, hkc * 2 * D], f16, tag=f"hh{c}")
                hh_eng.dma_start(
                    t[:], hh[:, c * hkc * 2 * D:(c + 1) * hkc * 2 * D]
                )
                hh_sb.append(t)
            wxt_sb = cst.tile([D, 4 * D], f16)
            io_eng.dma_start(wxt_sb[:], wxt[:])
            wht_sb = cst.tile([D, 4 * D], f32)
            io_eng.dma_start(wht_sb[:], wht[:])
            bias_sb = cst.tile([D, 4], f32)
            io_eng.dma_start(bias_sb[:], bias[:])
            xt_sb = cst.tile([D, rows], f16)
            io_eng.dma_start(xt_sb[:], xt[:])
            ct_sb = cst.tile([D, rows], f32)
            io_eng.dma_start(ct_sb[:], ct[:])

            def body(_iv=None):
                # outputs of m-block i are issued mid-way through m-block
                # i+1's stripe stream: by then they are computed, so they
                # never head-of-line block the adjacency DMA rings.
                pending = []

                for mbi in range(mb):
                    mbs = slice(mbi * mbw, (mbi + 1) * mbw)

                    # x-side gate matmuls first: no dependence on the
                    # adjacency stream, so they fill PE while DMA runs and
                    # leave only the h-side matmul on the m-block tail.
                    pgs = []
                    for gi in range(4):
                        pg = gatepsum.tile([D, mbw], f32, tag=f"pg{gi}")
                        nc.tensor.matmul(
                            pg[:],
                            wxt_sb[:, gi * D:(gi + 1) * D],
                            xt_sb[:, mbs],
                            start=True,
                            stop=False,
                        )
                        pgs.append(pg)

                    gps = gpsum.tile([128, mbw], f32, tag="gps")
                    for g in range(gd):
                        # first stripes of an m-block ride SP while the ACT
                        # ring drains the previous tail's output stores
                        eng = (dma_engs[0] if (mbi == 0 and g < 4)
                               else dma_engs[g % len(dma_engs)])
                        stripe = apool.tile([128, ktb * mbw], f16, tag="stripe")
                        rb = (mbi * gd + g) * 128
                        eng.dma_start(stripe[:], adjt[rb:rb + 128, :])
                        if g == 6:
                            for pi, (dst, tsb) in enumerate(pending):
                                dma_engs[pi % len(dma_engs)].dma_start(dst, tsb[:])
                            pending.clear()
                        for a in range(ktb):
                            kti = g * ktb + a
                            hc, ho = kti // hkc, kti % hkc
                            nc.tensor.matmul(
                                gps[:],
                                hh_sb[hc][:, ho * 2 * D:(ho + 1) * 2 * D],
                                stripe[:, a * mbw:(a + 1) * mbw],
                                start=(kti == 0),
                                stop=(kti == kt - 1),
                            )
                    # tail in two column-halves so PE/ACT/DVE stages pipeline
                    hw = mbw // 2
                    for h in range(2):
                        hs = slice(h * hw, (h + 1) * hw)       # within m-block
                        hg = slice(mbi * mbw + h * hw, mbi * mbw + (h + 1) * hw)
                        gtb = b64.tile([D, hw], f32, tag=f"gtb{h}")
                        nc.vector.tensor_copy(gtb[:], gps[0:D, hs])
                        nc.vector.tensor_add(gtb[:], gtb[:], gps[D:2 * D, hs])
                        gates = []
                        for gi, fname in enumerate(_GATE_FUNCS):
                            pg = pgs[gi]
                            nc.tensor.matmul(
                                pg[:, hs],
                                wht_sb[:, gi * D:(gi + 1) * D],
                                gtb[:],
                                start=False,
                                stop=(h == 1),
                                skip_group_check=True,
                            )
                            gate_sb = b64.tile([D, hw], f32, tag=f"gate{gi}{h}")
                            nc.scalar.activation(
                                gate_sb[:],
                                pg[:, hs],
                                getattr(act, fname),
                                bias=bias_sb[:, gi:gi + 1],
                            )
                            gates.append(gate_sb)
                        it_, ft_, ot_, ut_ = gates

                        t1 = b64.tile([D, hw], f32, tag=f"t1{h}")
                        nc.vector.tensor_mul(t1[:], ft_[:], ct_sb[:, hg])
                        t2 = b64.tile([D, hw], f32, tag=f"t2{h}")
                        nc.vector.tensor_mul(t2[:], it_[:], ut_[:])
                        ctn = b64.tile([D, hw], f32, tag=f"ctn{h}")
                        nc.vector.tensor_add(ctn[:], t1[:], t2[:])
                        tct = b64.tile([D, hw], f32, tag=f"tct{h}")
                        nc.scalar.activation(tct[:], ctn[:], act.Tanh)
                        htn = b64.tile([D, hw], f32, tag=f"htn{h}")
                        nc.vector.tensor_mul(htn[:], ot_[:], tct[:])
                        if mbi == mb - 1:
                            io_eng.dma_start(ct_out[:, hg], ctn[:])
                            io_eng.dma_start(ht_out[:, hg], htn[:])
                        else:
                            pending.append((ct_out[:, hg], ctn))
                            pending.append((ht_out[:, hg], htn))

            if repeat == 1:
                for _ in range(unroll):
                    body()
            else:
                # unroll>1 amortizes the per-trip InstAllEngineBarrier that
                # For_i inserts in its semaphore-reset block
                with tc.For_i(0, repeat, 1) as _i:
                    for _ in range(unroll):
                        body(_i)

    if split_waits:
        _split_excess_waits(nc)
    return nc


def make_in_maps(inputs, n=N, n_cores=N_CORES):
    """Host-side sharding + relayout. Returns per-core input dicts."""
    rows = n // n_cores
    kt = n // 128
    mb = rows // MBW
    gd = kt // KTB
    adj = np.asarray(inputs["adj_matrix"], dtype=np.float32)
    H = np.asarray(inputs["Ht_1"], dtype=np.float32)
    ht = np.asarray(inputs["ht"], dtype=np.float32)
    Ct_1 = np.asarray(inputs["Ct_1"], dtype=np.float32)

    # H = hi + lo to ~2^-22: the hi/lo fp16 pair is packed side by side
    # per k-tile ([128, kt*128]) so one matmul computes both products.
    Hh32 = H.astype(np.float16).astype(np.float32)
    packed = np.empty((128, kt, 2 * D), dtype=np.float16)
    packed[:, :, :D] = Hh32.reshape(kt, 128, D).transpose(1, 0, 2)
    packed[:, :, D:] = (H - Hh32).reshape(kt, 128, D).transpose(1, 0, 2)
    hh = np.ascontiguousarray(packed.reshape(128, kt * 2 * D))

    gate_w = ("Wxi", "Wxf", "Wxo", "Wxc")
    gate_h = ("Whi", "Whf", "Who", "Whc")
    wxt = np.concatenate(
        [np.asarray(inputs[g + "_w"], np.float32).T for g in gate_w], axis=1
    ).astype(np.float16)
    wht = np.concatenate(
        [np.asarray(inputs[g + "_w"], np.float32).T for g in gate_h], axis=1
    )
    # adj is shifted by -0.5 before the fp16 cast (halves quantization
    # error for uniform(0,1) entries). g = (adj-0.5)@H + 0.5*colsum(H)
    # broadcast over rows; the second term passes through the h-side
    # Linear as a per-feature constant, folded into the gate bias here.
    colsum = H.astype(np.float64).sum(axis=0)
    bias = np.stack(
        [
            np.asarray(inputs[gx + "_b"], np.float64)
            + np.asarray(inputs[gh + "_b"], np.float64)
            + 0.5 * (np.asarray(inputs[gh + "_w"], np.float64) @ colsum)
            for gx, gh in zip(gate_w, gate_h)
        ],
        axis=1,
    ).astype(np.float32)
    wxt = np.ascontiguousarray(wxt)
    wht = np.ascontiguousarray(wht)
    bias = np.ascontiguousarray(bias)

    in_maps = []
    for c in range(n_cores):
        rs = slice(c * rows, (c + 1) * rows)
        adjt_c = np.ascontiguousarray(adj[rs].T)
        adjt_c -= np.float32(0.5)
        a16 = adjt_c.astype(np.float16)          # [n, rows]
        # stripe-contiguous permute: [(mb gd) 128, ktb*mbw] where the row
        # block (mbi*gd+g)*128 holds k-rows (g*ktb .. )*128 interleaved as
        # [p, a, mj] -> flat [128, ktb*mbw] for m-cols mbi*mbw..+mbw.
        a5 = a16.reshape(gd, KTB, 128, mb, MBW).transpose(3, 0, 2, 1, 4)
        adjt_s = np.ascontiguousarray(a5.reshape(mb * gd * 128, KTB * MBW))
        in_maps.append(
            {
                "adjt": adjt_s,
                "hh": hh,
                "xt": np.ascontiguousarray(ht[rs].T).astype(np.float16),
                "ct": np.ascontiguousarray(Ct_1[rs].T),
                "wxt": wxt,
                "wht": wht,
                "bias": bias,
            }
        )
    return in_maps


def gather(results):
    Ht = np.concatenate([r["ht_out"].T for r in results], axis=0)
    Ct = np.concatenate([r["ct_out"].T for r in results], axis=0)
    return np.ascontiguousarray(Ht), np.ascontiguousarray(Ct)


_PROGRAM_CACHE = {}


def kernel(**inputs):
    from concourse.bass_utils import run_bass_kernel_spmd

    if "nc" not in _PROGRAM_CACHE:
        _PROGRAM_CACHE["nc"] = build()
    nc = _PROGRAM_CACHE["nc"]
    in_maps = make_in_maps(inputs)
    res = run_bass_kernel_spmd(nc, in_maps, list(range(N_CORES)))
    return gather(res.results)


# revision 15
# speedup vs baseline: 2.0059x; 1.2315x over previous
"""GST-LSTM cell (graph-conv LSTM) on 8 Trainium2 NeuronCores.

Computation (reference):
    g  = adj_matrix @ Ht_1                       # (N, H)  -- dominant cost
    i  = sigmoid(ht @ Wxi.T + bxi + g @ Whi.T + bhi)
    f  = sigmoid(ht @ Wxf.T + bxf + g @ Whf.T + bhf)
    o  = sigmoid(ht @ Wxo.T + bxo + g @ Who.T + bho)
    u  = tanh   (ht @ Wxc.T + bxc + g @ Whc.T + bhc)
    Ct = f * Ct_1 + i * u
    Ht = o * tanh(Ct)

Sharding: node dim (rows of adj, ht, Ct_1; output rows) split across the
8 cores; Ht_1 replicated. No collectives needed.

Device layout: everything is computed feature-major ([64, nodes] tiles)
so that the PE contraction dim lands on partitions without any on-device
transposes:
  - adj is transposed, shifted by -0.5, cast to fp16 and PERMUTED on the
    host into stripe-contiguous order: each [128 k-rows, KTB*512] stripe
    a DMA fetches is one contiguous 1 MiB region of DRAM. Stripe loads
    alternate between the two HWDGE queues (SP via nc.sync, ACT via
    nc.scalar) so descriptor generation and completion latencies overlap.
  - Ht_1 enters as an fp16 hi/lo pair packed side by side per k-tile
    ([128, kt*128]): one [128,128] stationary computes both products in a
    single matmul pass (PSUM rows 0:64 accumulate A@H_hi, rows 64:128
    A@H_lo). It is loaded in 8 chunks so the first matmul starts ~2 us in.
  - the eight 64x64 Linears run feature-major; the x-side (ht) matmuls
    are fp16 (negligible error) and hoisted to m-block start so only the
    fp32 h-side matmul + activation + gating sit on the tail. Biases
    enter via the ACT engine's per-partition bias operand.
  - const loads and output stores ride the Pool SWDGE queue so they never
    head-of-line block the adjacency stream on the HWDGE rings.

fp16 for the adj @ Ht_1 product keeps end-to-end relative error at the
~1.2e-2 level (fp32 PSUM accumulation) while halving HBM traffic of the
1 GiB adjacency stream, which is what the memory-bound regime rewards.
"""

import numpy as np

N = 16384
D = 64
N_CORES = 8
ROWS = N // N_CORES          # 2048 nodes per core
MBW = 512                    # m-block width (PE moving free dim / PSUM bank)
MB = ROWS // MBW             # 4 m-blocks per core
KT = N // 128                # 128 k-tiles of 128 contraction rows
KTB = 16                     # k-tiles fetched per DMA (2 MiB stripes)
GD = KT // KTB               # stripe DMAs per m-block
HHC = 8                      # hh load chunks

_GATE_FUNCS = ("Sigmoid", "Sigmoid", "Sigmoid", "Tanh")  # i, f, o, u


def _split_excess_waits(nc, max_waits=1):
    """Split >max_waits sem waits off instructions onto preceding NOPs.

    The walrus build here rejects instructions carrying more than a
    couple of sync waits ("Too many sync wait commands" from
    setupSyncWait during codegen). Tile's wait assignment doesn't know
    that limit; an NX engine executes its stream in order, so moving
    the excess waits onto same-engine NOPs directly before the
    instruction preserves ordering semantics with a legal encoding.
    """
    from concourse import mybir

    fn = nc.m.functions[0]
    for bb in fn.blocks:
        out = []
        for inst in bb.instructions:
            si = getattr(inst, "sync_info", None)
            if si is not None and si.on_wait and len(si.on_wait) > max_waits:
                waits = list(si.on_wait)
                spill, keep = waits[:-max_waits], waits[-max_waits:]
                for i in range(0, len(spill), max_waits):
                    nop = mybir.InstNoOp(
                        name=nc.get_next_instruction_name(),
                        sync_info=mybir.SyncInfo(
                            on_wait=spill[i:i + max_waits], on_update=[]
                        ),
                        bass_nofuse=True,
                        engine=inst.engine,
                    )
                    out.append(nop)
                si.on_wait = keep
            out.append(inst)
        bb.instructions[:] = out


def build(n=N, rows=ROWS, mbw=MBW, ktb=KTB, repeat=1, adj_bufs=4,
          split_waits=True, dual_queue=True, hoist_x=True, pool_io=True,
          unroll=1):
    """Build the per-core Bass program. All cores run the same program."""
    import concourse.bass as bass
    import concourse.mybir as mybir
    from concourse import tile

    dt = mybir.dt
    act = mybir.ActivationFunctionType
    f16, f32 = dt.float16, dt.float32

    kt = n // 128
    mb = rows // mbw
    gd = kt // ktb
    hkc = kt // HHC              # k-tiles per hh chunk

    nc = bass.Bass()
    # stripe-contiguous: row block (mbi*gd + g)*128 .. +128 is one stripe
    adjt = nc.declare_dram_parameter("adjt", [mb * gd * 128, ktb * mbw], f16,
                                     isOutput=False)
    # hh packs [H_hi | H_lo] fp16 side by side per k-tile: one [128,128]
    # stationary computes both products in a single matmul pass.
    hh = nc.declare_dram_parameter("hh", [128, kt * 2 * D], f16, isOutput=False)
    xt = nc.declare_dram_parameter("xt", [D, rows], f16, isOutput=False)
    ct = nc.declare_dram_parameter("ct", [D, rows], f32, isOutput=False)
    wxt = nc.declare_dram_parameter("wxt", [D, 4 * D], f16, isOutput=False)
    wht = nc.declare_dram_parameter("wht", [D, 4 * D], f32, isOutput=False)
    bias = nc.declare_dram_parameter("bias", [D, 4], f32, isOutput=False)
    ht_out = nc.declare_dram_parameter("ht_out", [D, rows], f32, isOutput=True)
    ct_out = nc.declare_dram_parameter("ct_out", [D, rows], f32, isOutput=True)

    dma_engs = []

    with tile.TileContext(nc) as tc:
        with (
            tc.tile_pool(name="const", bufs=1) as cst,
            tc.tile_pool(name="adj", bufs=adj_bufs) as apool,
            tc.tile_pool(name="b64", bufs=3) as b64,
            tc.tile_pool(name="gpsum", bufs=2, space="PSUM") as gpsum,
            tc.tile_pool(name="gatepsum", bufs=1, space="PSUM") as gatepsum,
        ):
            io_eng = nc.scalar if pool_io else nc.sync
            dma_engs = [nc.sync, nc.scalar] if dual_queue else [nc.sync]

            # hh in chunks on the ACT ring (so SP's first job is stripe 0),
            # consts on Pool
            hh_eng = dma_engs[-1]
            hh_sb = []
            for c in range(HHC):
                t = cst.tile([128, hkc * 2 * D], f16, tag=f"hh{c}")
                hh_eng.dma_start(
                    t[:], hh[:, c * hkc * 2 * D:(c + 1) * hkc * 2 * D]
                )
                hh_sb.append(t)
            wxt_sb = cst.tile([D, 4 * D], f16)
            io_eng.dma_start(wxt_sb[:], wxt[:])
            wht_sb = cst.tile([D, 4 * D], f32)
            io_eng.dma_start(wht_sb[:], wht[:])
            bias_sb = cst.tile([D, 4], f32)
            io_eng.dma_start(bias_sb[:], bias[:])
            xt_sb = cst.tile([D, rows], f16)
            io_eng.dma_start(xt_sb[:], xt[:])
            ct_sb = cst.tile([D, rows], f32)
            io_eng.dma_start(ct_sb[:], ct[:])

            def body(_iv=None):
                # outputs of m-block i are issued mid-way through m-block
                # i+1's stripe stream: by then they are computed, so they
                # never head-of-line block the adjacency DMA rings.
                pending = []

                for mbi in range(mb):
                    mbs = slice(mbi * mbw, (mbi + 1) * mbw)

                    # x-side gate matmuls first: no dependence on the
                    # adjacency stream, so they fill PE while DMA runs and
                    # leave only the h-side matmul on the m-block tail.
                    pgs = []
                    for gi in range(4):
                        pg = gatepsum.tile([D, mbw], f32, tag=f"pg{gi}")
                        nc.tensor.matmul(
                            pg[:],
                            wxt_sb[:, gi * D:(gi + 1) * D],
                            xt_sb[:, mbs],
                            start=True,
                            stop=False,
                        )
                        pgs.append(pg)

                    gps = gpsum.tile([128, mbw], f32, tag="gps")
                    for g in range(gd):
                        # first stripes of an m-block ride SP while the ACT
                        # ring drains the previous tail's output stores
                        eng = (dma_engs[0] if (mbi == 0 and g < 4)
                               else dma_engs[g % len(dma_engs)])
                        stripe = apool.tile([128, ktb * mbw], f16, tag="stripe")
                        rb = (mbi * gd + g) * 128
                        eng.dma_start(stripe[:], adjt[rb:rb + 128, :])
                        if g == 6:
                            for pi, (dst, tsb) in enumerate(pending):
                                dma_engs[pi % len(dma_engs)].dma_start(dst, tsb[:])
                            pending.clear()
                        for a in range(ktb):
                            kti = g * ktb + a
                            hc, ho = kti // hkc, kti % hkc
                            nc.tensor.matmul(
                                gps[:],
                                hh_sb[hc][:, ho * 2 * D:(ho + 1) * 2 * D],
                                stripe[:, a * mbw:(a + 1) * mbw],
                                start=(kti == 0),
                                stop=(kti == kt - 1),
                            )
                    # tail in two column-halves so PE/ACT/DVE stages pipeline
                    hw = mbw // 2
                    for h in range(2):
                        hs = slice(h * hw, (h + 1) * hw)       # within m-block
                        hg = slice(mbi * mbw + h * hw, mbi * mbw + (h + 1) * hw)
                        gtb = b64.tile([D, hw], f32, tag=f"gtb{h}")
                        nc.vector.tensor_copy(gtb[:], gps[0:D, hs])
                        nc.vector.tensor_add(gtb[:], gtb[:], gps[D:2 * D, hs])
                        gates = []
                        for gi, fname in enumerate(_GATE_FUNCS):
                            pg = pgs[gi]
                            nc.tensor.matmul(
                                pg[:, hs],
                                wht_sb[:, gi * D:(gi + 1) * D],
                                gtb[:],
                                start=False,
                                stop=(h == 1),
                                skip_group_check=True,
                            )
                            gate_sb = b64.tile([D, hw], f32, tag=f"gate{gi}{h}")
                            nc.scalar.activation(
                                gate_sb[:],
                                pg[:, hs],
                                getattr(act, fname),
                                bias=bias_sb[:, gi:gi + 1],
                            )
                            gates.append(gate_sb)
                        it_, ft_, ot_, ut_ = gates

                        t1 = b64.tile([D, hw], f32, tag=f"t1{h}")
                        nc.vector.tensor_mul(t1[:], ft_[:], ct_sb[:, hg])
                        t2 = b64.tile([D, hw], f32, tag=f"t2{h}")
                        nc.vector.tensor_mul(t2[:], it_[:], ut_[:])
                        ctn = b64.tile([D, hw], f32, tag=f"ctn{h}")
                        nc.vector.tensor_add(ctn[:], t1[:], t2[:])
                        tct = b64.tile([D, hw], f32, tag=f"tct{h}")
                        nc.scalar.activation(tct[:], ctn[:], act.Tanh)
                        htn = b64.tile([D, hw], f32, tag=f"htn{h}")
                        nc.vector.tensor_mul(htn[:], ot_[:], tct[:])
                        if mbi == mb - 1:
                            io_eng.dma_start(ct_out[:, hg], ctn[:])
                            io_eng.dma_start(ht_out[:, hg], htn[:])
                        else:
                            pending.append((ct_out[:, hg], ctn))
                            pending.append((ht_out[:, hg], htn))

            if repeat == 1:
                for _ in range(unroll):
                    body()
            else:
                # unroll>1 amortizes the per-trip InstAllEngineBarrier that
                # For_i inserts in its semaphore-reset block
                with tc.For_i(0, repeat, 1) as _i:
                    for _ in range(unroll):
                        body(_i)

    if split_waits:
        _split_excess_waits(nc)
    return nc


def make_in_maps(inputs, n=N, n_cores=N_CORES):
    """Host-side sharding + relayout. Returns per-core input dicts."""
    rows = n // n_cores
    kt = n // 128
    mb = rows // MBW
    gd = kt // KTB
    adj = np.asarray(inputs["adj_matrix"], dtype=np.float32)
    H = np.asarray(inputs["Ht_1"], dtype=np.float32)
    ht = np.asarray(inputs["ht"], dtype=np.float32)
    Ct_1 = np.asarray(inputs["Ct_1"], dtype=np.float32)

    # H = hi + lo to ~2^-22: the hi/lo fp16 pair is packed side by side
    # per k-tile ([128, kt*128]) so one matmul computes both products.
    Hh32 = H.astype(np.float16).astype(np.float32)
    packed = np.empty((128, kt, 2 * D), dtype=np.float16)
    packed[:, :, :D] = Hh32.reshape(kt, 128, D).transpose(1, 0, 2)
    packed[:, :, D:] = (H - Hh32).reshape(kt, 128, D).transpose(1, 0, 2)
    hh = np.ascontiguousarray(packed.reshape(128, kt * 2 * D))

    gate_w = ("Wxi", "Wxf", "Wxo", "Wxc")
    gate_h = ("Whi", "Whf", "Who", "Whc")
    wxt = np.concatenate(
        [np.asarray(inputs[g + "_w"], np.float32).T for g in gate_w], axis=1
    ).astype(np.float16)
    wht = np.concatenate(
        [np.asarray(inputs[g + "_w"], np.float32).T for g in gate_h], axis=1
    )
    # adj is shifted by -0.5 before the fp16 cast (halves quantization
    # error for uniform(0,1) entries). g = (adj-0.5)@H + 0.5*colsum(H)
    # broadcast over rows; the second term passes through the h-side
    # Linear as a per-feature constant, folded into the gate bias here.
    colsum = H.astype(np.float64).sum(axis=0)
    bias = np.stack(
        [
            np.asarray(inputs[gx + "_b"], np.float64)
            + np.asarray(inputs[gh + "_b"], np.float64)
            + 0.5 * (np.asarray(inputs[gh + "_w"], np.float64) @ colsum)
            for gx, gh in zip(gate_w, gate_h)
        ],
        axis=1,
    ).astype(np.float32)
    wxt = np.ascontiguousarray(wxt)
    wht = np.ascontiguousarray(wht)
    bias = np.ascontiguousarray(bias)

    in_maps = []
    for c in range(n_cores):
        rs = slice(c * rows, (c + 1) * rows)
        adjt_c = np.ascontiguousarray(adj[rs].T)
        adjt_c -= np.float32(0.5)
        a16 = adjt_c.astype(np.float16)          # [n, rows]
        # stripe-contiguous permute: [(mb gd) 128, ktb*mbw] where the row
        # block (mbi*gd+g)*128 holds k-rows (g*ktb .. )*128 interleaved as
        # [p, a, mj] -> flat [128, ktb*mbw] for m-cols mbi*mbw..+mbw.
        a5 = a16.reshape(gd, KTB, 128, mb, MBW).transpose(3, 0, 2, 1, 4)
        adjt_s = np.ascontiguousarray(a5.reshape(mb * gd * 128, KTB * MBW))
        in_maps.append(
            {
                "adjt": adjt_s,
                "hh": hh,
                "xt": np.ascontiguousarray(ht[rs].T).astype(np.float16),
                "ct": np.ascontiguousarray(Ct_1[rs].T),
                "wxt": wxt,
                "wht": wht,
                "bias": bias,
            }
        )
    return in_maps


def gather(results):
    Ht = np.concatenate([r["ht_out"].T for r in results], axis=0)
    Ct = np.concatenate([r["ct_out"].T for r in results], axis=0)
    return np.ascontiguousarray(Ht), np.ascontiguousarray(Ct)


_PROGRAM_CACHE = {}


def kernel(**inputs):
    from concourse.bass_utils import run_bass_kernel_spmd

    if "nc" not in _PROGRAM_CACHE:
        _PROGRAM_CACHE["nc"] = build()
    nc = _PROGRAM_CACHE["nc"]
    in_maps = make_in_maps(inputs)
    res = run_bass_kernel_spmd(nc, in_maps, list(range(N_CORES)))
    return gather(res.results)


# revision 19
# speedup vs baseline: 2.1427x; 1.0682x over previous
"""GST-LSTM cell (graph-conv LSTM) on 8 Trainium2 NeuronCores.

Computation (reference):
    g  = adj_matrix @ Ht_1                       # (N, H)  -- dominant cost
    i  = sigmoid(ht @ Wxi.T + bxi + g @ Whi.T + bhi)
    f  = sigmoid(ht @ Wxf.T + bxf + g @ Whf.T + bhf)
    o  = sigmoid(ht @ Wxo.T + bxo + g @ Who.T + bho)
    u  = tanh   (ht @ Wxc.T + bxc + g @ Whc.T + bhc)
    Ct = f * Ct_1 + i * u
    Ht = o * tanh(Ct)

Sharding: node dim (rows of adj, ht, Ct_1; output rows) split across the
8 cores; Ht_1 replicated. No collectives needed.

Device layout: everything is computed feature-major ([64, nodes] tiles)
so that the PE contraction dim lands on partitions without any on-device
transposes:
  - adj is transposed, shifted by -0.5, cast to fp16 and PERMUTED on the
    host into stripe-contiguous order: each [128 k-rows, KTB*512] stripe
    a DMA fetches is one contiguous 2 MiB region of DRAM (2 MiB beats
    1 MiB/512 KiB by amortizing per-DMA overhead: 259us -> 210us/iter).
    Stripe loads alternate between the two HWDGE queues (SP via nc.sync,
    ACT via nc.scalar) so descriptor generation and completion latencies
    overlap.
  - Ht_1 enters as an fp16 hi/lo pair packed side by side per k-tile
    ([128, kt*128]): one [128,128] stationary computes both products in a
    single matmul pass (PSUM rows 0:64 accumulate A@H_hi, rows 64:128
    A@H_lo). It is loaded in 8 chunks so the first matmul starts ~2 us in.
  - the eight 64x64 Linears run feature-major; the x-side (ht) matmuls
    are fp16 (negligible error) and hoisted to m-block start so only the
    fp32 h-side matmul + activation + gating sit on the tail. Biases
    enter via the ACT engine's per-partition bias operand.
  - const loads and output stores ride the Pool SWDGE queue so they never
    head-of-line block the adjacency stream on the HWDGE rings.

fp16 for the adj @ Ht_1 product keeps end-to-end relative error at the
~1.2e-2 level (fp32 PSUM accumulation) while halving HBM traffic of the
1 GiB adjacency stream, which is what the memory-bound regime rewards.
"""

import numpy as np

N = 16384
D = 64
N_CORES = 8
ROWS = N // N_CORES          # 2048 nodes per core
MBW = 512                    # m-block width (PE moving free dim / PSUM bank)
MB = ROWS // MBW             # 4 m-blocks per core
KT = N // 128                # 128 k-tiles of 128 contraction rows
KTB = 32                     # k-tiles fetched per DMA (4 MiB stripes)
GD = KT // KTB               # stripe DMAs per m-block
HHC = 8                      # hh load chunks

_GATE_FUNCS = ("Sigmoid", "Sigmoid", "Sigmoid", "Tanh")  # i, f, o, u


def _split_excess_waits(nc, max_waits=1):
    """Split >max_waits sem waits off instructions onto preceding NOPs.

    The walrus build here rejects instructions carrying more than a
    couple of sync waits ("Too many sync wait commands" from
    setupSyncWait during codegen). Tile's wait assignment doesn't know
    that limit; an NX engine executes its stream in order, so moving
    the excess waits onto same-engine NOPs directly before the
    instruction preserves ordering semantics with a legal encoding.
    """
    from concourse import mybir

    fn = nc.m.functions[0]
    for bb in fn.blocks:
        out = []
        for inst in bb.instructions:
            si = getattr(inst, "sync_info", None)
            if si is not None and si.on_wait and len(si.on_wait) > max_waits:
                waits = list(si.on_wait)
                spill, keep = waits[:-max_waits], waits[-max_waits:]
                for i in range(0, len(spill), max_waits):
                    nop = mybir.InstNoOp(
                        name=nc.get_next_instruction_name(),
                        sync_info=mybir.SyncInfo(
                            on_wait=spill[i:i + max_waits], on_update=[]
                        ),
                        bass_nofuse=True,
                        engine=inst.engine,
                    )
                    out.append(nop)
                si.on_wait = keep
            out.append(inst)
        bb.instructions[:] = out


def build(n=N, rows=ROWS, mbw=MBW, ktb=KTB, repeat=1, adj_bufs=3,
          split_waits=True, dual_queue=True, hoist_x=True, pool_io=True,
          unroll=1):
    """Build the per-core Bass program. All cores run the same program."""
    import concourse.bass as bass
    import concourse.mybir as mybir
    from concourse import tile

    dt = mybir.dt
    act = mybir.ActivationFunctionType
    f16, f32 = dt.float16, dt.float32

    kt = n // 128
    mb = rows // mbw
    gd = kt // ktb
    hkc = kt // HHC              # k-tiles per hh chunk

    nc = bass.Bass()
    # stripe-contiguous: row block (mbi*gd + g)*128 .. +128 is one stripe
    adjt = nc.declare_dram_parameter("adjt", [mb * gd * 128, ktb * mbw], f16,
                                     isOutput=False)
    # hh packs [H_hi | H_lo] fp16 side by side per k-tile: one [128,128]
    # stationary computes both products in a single matmul pass.
    hh = nc.declare_dram_parameter("hh", [128, kt * 2 * D], f16, isOutput=False)
    xt = nc.declare_dram_parameter("xt", [D, rows], f16, isOutput=False)
    ct = nc.declare_dram_parameter("ct", [D, rows], f32, isOutput=False)
    wxt = nc.declare_dram_parameter("wxt", [D, 4 * D], f16, isOutput=False)
    wht = nc.declare_dram_parameter("wht", [D, 4 * D], f32, isOutput=False)
    bias = nc.declare_dram_parameter("bias", [D, 4], f32, isOutput=False)
    ht_out = nc.declare_dram_parameter("ht_out", [D, rows], f32, isOutput=True)
    ct_out = nc.declare_dram_parameter("ct_out", [D, rows], f32, isOutput=True)

    dma_engs = []

    with tile.TileContext(nc) as tc:
        with (
            tc.tile_pool(name="const", bufs=1) as cst,
            tc.tile_pool(name="adj", bufs=adj_bufs) as apool,
            tc.tile_pool(name="b64", bufs=3) as b64,
            tc.tile_pool(name="gpsum", bufs=2, space="PSUM") as gpsum,
            tc.tile_pool(name="gatepsum", bufs=1, space="PSUM") as gatepsum,
        ):
            io_eng = nc.scalar if pool_io else nc.sync
            dma_engs = [nc.sync, nc.scalar] if dual_queue else [nc.sync]

            # hh in chunks on the ACT ring (so SP's first job is stripe 0),
            # consts on Pool
            hh_eng = dma_engs[-1]
            hh_sb = []
            for c in range(HHC):
                t = cst.tile([128, hkc * 2 * D], f16, tag=f"hh{c}")
                hh_eng.dma_start(
                    t[:], hh[:, c * hkc * 2 * D:(c + 1) * hkc * 2 * D]
                )
                hh_sb.append(t)
            wxt_sb = cst.tile([D, 4 * D], f16)
            io_eng.dma_start(wxt_sb[:], wxt[:])
            wht_sb = cst.tile([D, 4 * D], f32)
            io_eng.dma_start(wht_sb[:], wht[:])
            bias_sb = cst.tile([D, 4], f32)
            io_eng.dma_start(bias_sb[:], bias[:])
            xt_sb = cst.tile([D, rows], f16)
            io_eng.dma_start(xt_sb[:], xt[:])
            ct_sb = cst.tile([D, rows], f32)
            io_eng.dma_start(ct_sb[:], ct[:])

            def body(_iv=None):
                # outputs of m-block i are issued mid-way through m-block
                # i+1's stripe stream: by then they are computed, so they
                # never head-of-line block the adjacency DMA rings.
                pending = []

                for mbi in range(mb):
                    mbs = slice(mbi * mbw, (mbi + 1) * mbw)

                    # x-side gate matmuls first: no dependence on the
                    # adjacency stream, so they fill PE while DMA runs and
                    # leave only the h-side matmul on the m-block tail.
                    pgs = []
                    for gi in range(4):
                        pg = gatepsum.tile([D, mbw], f32, tag=f"pg{gi}")
                        nc.tensor.matmul(
                            pg[:],
                            wxt_sb[:, gi * D:(gi + 1) * D],
                            xt_sb[:, mbs],
                            start=True,
                            stop=False,
                        )
                        pgs.append(pg)

                    flush_g = min(6, gd - 2)
                    gps = gpsum.tile([128, mbw], f32, tag="gps")
                    for g in range(gd):
                        # first stripes of an m-block ride SP while the ACT
                        # ring drains the previous tail's output stores
                        eng = (dma_engs[0] if (mbi == 0 and g < min(4, gd // 2))
                               else dma_engs[g % len(dma_engs)])
                        stripe = apool.tile([128, ktb * mbw], f16, tag="stripe")
                        rb = (mbi * gd + g) * 128
                        eng.dma_start(stripe[:], adjt[rb:rb + 128, :])
                        if g == flush_g:
                            for pi, (dst, tsb) in enumerate(pending):
                                dma_engs[pi % len(dma_engs)].dma_start(dst, tsb[:])
                            pending.clear()
                        for a in range(ktb):
                            kti = g * ktb + a
                            hc, ho = kti // hkc, kti % hkc
                            nc.tensor.matmul(
                                gps[:],
                                hh_sb[hc][:, ho * 2 * D:(ho + 1) * 2 * D],
                                stripe[:, a * mbw:(a + 1) * mbw],
                                start=(kti == 0),
                                stop=(kti == kt - 1),
                            )
                    # tail in two column-halves so PE/ACT/DVE stages pipeline
                    hw = mbw // 2
                    for h in range(2):
                        hs = slice(h * hw, (h + 1) * hw)       # within m-block
                        hg = slice(mbi * mbw + h * hw, mbi * mbw + (h + 1) * hw)
                        gtb = b64.tile([D, hw], f32, tag=f"gtb{h}")
                        nc.vector.tensor_copy(gtb[:], gps[0:D, hs])
                        nc.vector.tensor_add(gtb[:], gtb[:], gps[D:2 * D, hs])
                        gates = []
                        for gi, fname in enumerate(_GATE_FUNCS):
                            pg = pgs[gi]
                            nc.tensor.matmul(
                                pg[:, hs],
                                wht_sb[:, gi * D:(gi + 1) * D],
                                gtb[:],
                                start=False,
                                stop=(h == 1),
                                skip_group_check=True,
                            )
                            gate_sb = b64.tile([D, hw], f32, tag=f"gate{gi}{h}")
                            nc.scalar.activation(
                                gate_sb[:],
                                pg[:, hs],
                                getattr(act, fname),
                                bias=bias_sb[:, gi:gi + 1],
                            )
                            gates.append(gate_sb)
                        it_, ft_, ot_, ut_ = gates

                        t1 = b64.tile([D, hw], f32, tag=f"t1{h}")
                        nc.vector.tensor_mul(t1[:], ft_[:], ct_sb[:, hg])
                        t2 = b64.tile([D, hw], f32, tag=f"t2{h}")
                        nc.vector.tensor_mul(t2[:], it_[:], ut_[:])
                        ctn = b64.tile([D, hw], f32, tag=f"ctn{h}")
                        nc.vector.tensor_add(ctn[:], t1[:], t2[:])
                        tct = b64.tile([D, hw], f32, tag=f"tct{h}")
                        nc.scalar.activation(tct[:], ctn[:], act.Tanh)
                        htn = b64.tile([D, hw], f32, tag=f"htn{h}")
                        nc.vector.tensor_mul(htn[:], ot_[:], tct[:])
                        if mbi == mb - 1:
                            io_eng.dma_start(ct_out[:, hg], ctn[:])
                            io_eng.dma_start(ht_out[:, hg], htn[:])
                        else:
                            pending.append((ct_out[:, hg], ctn))
                            pending.append((ht_out[:, hg], htn))

            if repeat == 1:
                for _ in range(unroll):
                    body()
            else:
                # unroll>1 amortizes the per-trip InstAllEngineBarrier that
                # For_i inserts in its semaphore-reset block
                with tc.For_i(0, repeat, 1) as _i:
                    for _ in range(unroll):
                        body(_i)

    if split_waits:
        _split_excess_waits(nc)
    return nc


def make_in_maps(inputs, n=N, n_cores=N_CORES):
    """Host-side sharding + relayout. Returns per-core input dicts."""
    rows = n // n_cores
    kt = n // 128
    mb = rows // MBW
    gd = kt // KTB
    adj = np.asarray(inputs["adj_matrix"], dtype=np.float32)
    H = np.asarray(inputs["Ht_1"], dtype=np.float32)
    ht = np.asarray(inputs["ht"], dtype=np.float32)
    Ct_1 = np.asarray(inputs["Ct_1"], dtype=np.float32)

    # H = hi + lo to ~2^-22: the hi/lo fp16 pair is packed side by side
    # per k-tile ([128, kt*128]) so one matmul computes both products.
    Hh32 = H.astype(np.float16).astype(np.float32)
    packed = np.empty((128, kt, 2 * D), dtype=np.float16)
    packed[:, :, :D] = Hh32.reshape(kt, 128, D).transpose(1, 0, 2)
    packed[:, :, D:] = (H - Hh32).reshape(kt, 128, D).transpose(1, 0, 2)
    hh = np.ascontiguousarray(packed.reshape(128, kt * 2 * D))

    gate_w = ("Wxi", "Wxf", "Wxo", "Wxc")
    gate_h = ("Whi", "Whf", "Who", "Whc")
    wxt = np.concatenate(
        [np.asarray(inputs[g + "_w"], np.float32).T for g in gate_w], axis=1
    ).astype(np.float16)
    wht = np.concatenate(
        [np.asarray(inputs[g + "_w"], np.float32).T for g in gate_h], axis=1
    )
    # adj is shifted by -0.5 before the fp16 cast (halves quantization
    # error for uniform(0,1) entries). g = (adj-0.5)@H + 0.5*colsum(H)
    # broadcast over rows; the second term passes through the h-side
    # Linear as a per-feature constant, folded into the gate bias here.
    colsum = H.astype(np.float64).sum(axis=0)
    bias = np.stack(
        [
            np.asarray(inputs[gx + "_b"], np.float64)
            + np.asarray(inputs[gh + "_b"], np.float64)
            + 0.5 * (np.asarray(inputs[gh + "_w"], np.float64) @ colsum)
            for gx, gh in zip(gate_w, gate_h)
        ],
        axis=1,
    ).astype(np.float32)
    wxt = np.ascontiguousarray(wxt)
    wht = np.ascontiguousarray(wht)
    bias = np.ascontiguousarray(bias)

    in_maps = []
    for c in range(n_cores):
        rs = slice(c * rows, (c + 1) * rows)
        adjt_c = np.ascontiguousarray(adj[rs].T)
        adjt_c -= np.float32(0.5)
        a16 = adjt_c.astype(np.float16)          # [n, rows]
        # stripe-contiguous permute: [(mb gd) 128, ktb*mbw] where the row
        # block (mbi*gd+g)*128 holds k-rows (g*ktb .. )*128 interleaved as
        # [p, a, mj] -> flat [128, ktb*mbw] for m-cols mbi*mbw..+mbw.
        a5 = a16.reshape(gd, KTB, 128, mb, MBW).transpose(3, 0, 2, 1, 4)
        adjt_s = np.ascontiguousarray(a5.reshape(mb * gd * 128, KTB * MBW))
        in_maps.append(
            {
                "adjt": adjt_s,
                "hh": hh,
                "xt": np.ascontiguousarray(ht[rs].T).astype(np.float16),
                "ct": np.ascontiguousarray(Ct_1[rs].T),
                "wxt": wxt,
                "wht": wht,
                "bias": bias,
            }
        )
    return in_maps


def gather(results):
    Ht = np.concatenate([r["ht_out"].T for r in results], axis=0)
    Ct = np.concatenate([r["ct_out"].T for r in results], axis=0)
    return np.ascontiguousarray(Ht), np.ascontiguousarray(Ct)


_PROGRAM_CACHE = {}


def kernel(**inputs):
    from concourse.bass_utils import run_bass_kernel_spmd

    if "nc" not in _PROGRAM_CACHE:
        _PROGRAM_CACHE["nc"] = build()
    nc = _PROGRAM_CACHE["nc"]
    in_maps = make_in_maps(inputs)
    res = run_bass_kernel_spmd(nc, in_maps, list(range(N_CORES)))
    return gather(res.results)
